# revision 60
# baseline (speedup 1.0000x reference)
"""GQA attention kernel for 8 Trainium2 NeuronCores.

Sharding: batch x head-group. Core c handles batch b = c // 4 and head
group g = c % 4 (8 q heads 8g..8g+7, kv heads 2g, 2g+1). Each core
computes a partial output  attn_out_g[b] @ w_out[rows of g]  and the
host sums the 4 partials per batch.

On-chip layout is fully transposed: x^T arrives via bf16 XBAR DMA
transpose, q^T/k^T come straight out of the QKV^T projection, scores
are computed as S^T = K @ Q^T (softmax over the partition dim).

PE cost in TimelineSim is (output free-size) cycles per matmul, so the
kernel minimizes total matmul output columns:
 - scores: two heads as row-tile pair (contraction 64 each)
 - PV: per-head lhsT [V_h | ones64] (M=128) so the softmax denominator
   accumulates FREE on the 64 partitions opposite the PV output
   (replaces the old dedicated ones-matmul pair, -512 cyc/key-tile);
   normalization is two partition-crossed reciprocals + two multiplies
   (DVE ops may cross partition offsets between in and out).
 - causal trim: on diagonal key-tiles the scores/exp/PV q-range starts
   at the diagonal (N = 512-128r), with a single [128,128] tri mask.
Phases interleave (A0 A1 B0 A2 B1 A3 B2 C0 B3 C1 C2 C3); B-stream
tails (PV drain + norm) are pipelined behind the next stream's head
with a per-stream scores->PV depth (5/4 past wave 0) so a stream's
first PV isn't gated on the previous stream's psum release, and the
epilogue covers the last norm chain with C(3) j=0..2 matmuls.
Weights load as a few large DMAs ordered by first use (transpose and
copy runs kept contiguous - the scheduler serializes at every
transpose<->copy boundary); x^T transposes are per-slab rounds so wave
w's columns are resident in time; junk warm-up matmuls hold the PE
p-state at full clock through the initial DMA wait; output rows go out
as single [128,2048] bf16 DMAs from the idle GpSimd queue (final row
per-chunk on HWDGE to shorten the drain tail).
"""

import numpy as np
import ml_dtypes

B, T, D = 2, 2048, 2048
H, KVH, HD = 32, 8, 64
KVD = KVH * HD  # 512
NCORES = 8
SCALE = 1.0 / np.sqrt(HD)

_CACHE = {}


def _build():
    import concourse.bass as bass
    import concourse.mybir as mybir
    import concourse.tile as tile
    from concourse import bacc

    f32 = mybir.dt.float32
    bf16 = mybir.dt.bfloat16
    AF = mybir.ActivationFunctionType
    OP = mybir.AluOpType

    nc = bacc.Bacc("TRN2", target_bir_lowering=False, debug=False)

    xb = nc.dram_tensor("xb", [T, D], bf16, kind="ExternalInput")
    wk = nc.dram_tensor("wk", [128, 16 * 128], bf16, kind="ExternalInput")
    wq = nc.dram_tensor("wq", [128, 16 * 512], bf16, kind="ExternalInput")
    wv = nc.dram_tensor("wv", [128, 16 * 128], bf16, kind="ExternalInput")
    wo = nc.dram_tensor("wo", [128, 4 * 2048], bf16, kind="ExternalInput")
    sinT = nc.dram_tensor("sinT", [128, T], bf16, kind="ExternalInput")
    cosT = nc.dram_tensor("cosT", [128, T], bf16, kind="ExternalInput")
    perm = nc.dram_tensor("perm", [128, 128], bf16, kind="ExternalInput")
    ident128 = nc.dram_tensor("ident128", [128, 128], bf16, kind="ExternalInput")
    masks = nc.dram_tensor("masks", [128, 128], bf16, kind="ExternalInput")
    outp = nc.dram_tensor("outp", [T, D], bf16, kind="ExternalOutput")

    DT = D // 128   # 16 d-tiles
    SLAB = 512

    with tile.TileContext(nc) as tc:
        with (
            tc.tile_pool(name="const", bufs=1) as cpool,
            tc.tile_pool(name="resid", bufs=1) as rpool,
            tc.tile_pool(name="pr", bufs=3) as pr,
            tc.tile_pool(name="pp", bufs=10) as pp,
            tc.tile_pool(name="pn", bufs=3) as pn,
            tc.tile_pool(name="pc", bufs=3) as pc,
            tc.tile_pool(name="ps512", bufs=2, space="PSUM") as ps512,
            tc.tile_pool(name="ps_sc", bufs=2, space="PSUM") as ps_sc,
            tc.tile_pool(name="ps_pvd", bufs=1, space="PSUM") as ps_pvd,
        ):
            # ---- resident tiles ----
            wk_sb = cpool.tile([128, 16 * 128], bf16, tag="wk")
            wq_sb = cpool.tile([128, 16 * 512], bf16, tag="wq")
            wv_sb = cpool.tile([128, 16 * 128], bf16, tag="wv")
            wo_sb = cpool.tile([128, 4 * 2048], bf16, tag="wo")
            sin_sb = cpool.tile([128, T], bf16, tag="sin")
            cos_sb = cpool.tile([128, T], bf16, tag="cos")
            perm_sb = cpool.tile([128, 128], bf16, tag="perm")
            id128_sb = cpool.tile([128, 128], bf16, tag="id128")
            mask_sb = cpool.tile([128, 128], bf16, tag="mask")

            xT = [rpool.tile([128, T], bf16, tag=f"xT{d}", name=f"xT{d}") for d in range(DT)]
            # qT tiles j=0..3: partitions 0:64 head j, 64:128 head j+4
            # kT = qkT[4]: partitions 0:64 kv head 2g, 64:128 kv head 2g+1
            qkT = [rpool.tile([128, T], bf16, tag=f"qkT{e}", name=f"qkT{e}") for e in range(5)]
            # vnat[kt]: [128 keys, 192]: cols 0:64 head-A v dims, 64:128
            # ones, 128:192 head-B v dims. lhsT [V_A|ones] = cols 0:128,
            # [ones|V_B] = cols 64:192 -> denominator rides the PV matmul.
            vnat = [rpool.tile([128, 192], bf16, tag=f"vn{k}", name=f"vn{k}") for k in range(16)]
            attnT = [rpool.tile([128, T], bf16, tag=f"attnT{j}", name=f"attnT{j}") for j in range(4)]

            # ---- input DMAs, ordered by first use ----
            # wq is laid out e-major ([e, d, 128] along columns) so the
            # e=0 blob loads as one contiguous 512KB DMA before the x
            # transposes; sin/cos slab-0 columns come right after so the
            # first rope chain isn't gated on the full tables.
            # PE warmup: junk matmuls bridge the initial DMA wait so the
            # p-state ramp reaches full clock before real work arrives.
            junk = cpool.tile([128, 512], bf16, tag="junk")
            nc.gpsimd.memset(junk[:], 0.0)
            for _ in range(19):
                jp = ps512.tile([128, 512], f32, tag="w")
                nc.tensor.matmul(jp[:], junk[:, 0:128], junk[:], start=True, stop=True)

            nc.sync.dma_start(wk_sb[:], wk[:])
            nc.sync.dma_start(wq_sb[:, 0:2048], wq[:, 0:2048])
            nc.sync.dma_start(perm_sb[:], perm[:])
            nc.sync.dma_start(sin_sb[:, 0:SLAB], sinT[:, 0:SLAB])
            nc.sync.dma_start(cos_sb[:, 0:SLAB], cosT[:, 0:SLAB])
            for d in range(DT):
                nc.sync.dma_start_transpose(
                    xT[d][:, 0:SLAB], xb[0:SLAB, d * 128:(d + 1) * 128])
            nc.sync.dma_start(wv_sb[:], wv[:])
            nc.sync.dma_start(id128_sb[:], ident128[:])
            nc.sync.dma_start(mask_sb[:], masks[:])
            nc.sync.dma_start(wq_sb[:, 2048:], wq[:, 2048:])
            nc.sync.dma_start(sin_sb[:, SLAB:], sinT[:, SLAB:])
            nc.sync.dma_start(cos_sb[:, SLAB:], cosT[:, SLAB:])
            for k in range(16):
                nc.gpsimd.memset(vnat[k][:, 64:128], 1.0)
            for d in range(DT):
                nc.sync.dma_start_transpose(
                    xT[d][:, SLAB:2 * SLAB],
                    xb[SLAB:2 * SLAB, d * 128:(d + 1) * 128])
            for d in range(DT):
                nc.sync.dma_start_transpose(
                    xT[d][:, 2 * SLAB:T], xb[2 * SLAB:T, d * 128:(d + 1) * 128])
            nc.sync.dma_start(wo_sb[:], wo[:])

            # ================= Phase A: projections (one slab) ==========
            def emit_A_e(s, e):
                sl = slice(s * SLAB, (s + 1) * SLAB)
                if True:
                    acc = ps512.tile([128, SLAB], f32, tag="w")
                    for d in range(DT):
                        if e == 4:
                            lhsT = wk_sb[:, d * 128:(d + 1) * 128]
                        elif e == 5:
                            lhsT = wv_sb[:, d * 128:(d + 1) * 128]
                        else:
                            wcol = e * 2048 + d * 128
                            lhsT = wq_sb[:, wcol:wcol + 128]
                        nc.tensor.matmul(
                            acc[:], lhsT, xT[d][:, sl],
                            start=(d == 0), stop=(d == DT - 1),
                        )
                    raw = pr.tile([128, SLAB], bf16, tag="raw")
                    with tc.high_priority():
                        nc.vector.tensor_copy(raw[:], acc[:])
                    if e == 5:
                        # vT -> PE transpose -> v natural layout
                        for i in range(4):
                            kt = 4 * s + i
                            vtp = ps512.tile([128, 128], bf16, tag="w")
                            nc.tensor.transpose(
                                vtp[:], raw[:, i * 128:(i + 1) * 128], id128_sb[:])
                            with tc.high_priority():
                                nc.vector.tensor_copy(vnat[kt][:, 0:64], vtp[:, 0:64])
                                nc.vector.tensor_copy(vnat[kt][:, 128:192], vtp[:, 64:128])
                        return
                    # rotate-half matmul reuses the acc tile in place (the
                    # raw copy has evacuated it), saving a psum slot
                    nc.tensor.matmul(acc[:], perm_sb[:], raw[:], start=True, stop=True)
                    with tc.high_priority():
                        m2 = pr.tile([128, SLAB], bf16, tag="m2")
                        nc.vector.tensor_tensor(m2[:], raw[:], cos_sb[:, sl], OP.mult)
                        m1 = pr.tile([128, SLAB], bf16, tag="m1")
                        nc.vector.tensor_tensor(m1[:], acc[:], sin_sb[:, sl], OP.mult)
                        nc.vector.tensor_tensor(qkT[e][:, sl], m1[:], m2[:], OP.add)

            # ================= Phase B: attention (one q-slab) ==========
            DEPTH = 3  # scores/exp run this many kt ahead of PV

            def emit_pv(stv, k0):
                pvdA, pvdB, probs, nkt, qs, j = stv[:6]
                p0 = probs.pop(k0)
                r = k0 - 4 * qs
                q0 = 128 * r if r > 0 else 0
                st, sp = (k0 == 0), (k0 == nkt - 1)
                # PV per head, M=128: [V_h | ones] -> PV on one partition
                # half, denominator broadcast on the other half, free.
                nc.tensor.matmul(
                    pvdA[:, q0:512], vnat[k0][:, 0:128],
                    p0[:, q0:512], start=st, stop=sp)
                nc.tensor.matmul(
                    pvdB[:, q0:512], vnat[k0][:, 64:192],
                    p0[:, 512 + q0:1024], start=st, stop=sp)

            def emit_B_head(qs, j):
                """Scores/exp for all kt + PV up to k0 = nkt-1-DEPTH."""
                nkt = 4 * qs + 4
                # pvdA: 0:64 = PV head j, 64:128 = denom(j) broadcast
                # pvdB: 0:64 = denom(j+4) broadcast, 64:128 = PV head j+4
                pvdA = ps_pvd.tile([128, 512], f32, tag="pvA")
                pvdB = ps_pvd.tile([128, 512], f32, tag="pvB")
                depth = 5 if (j == 0 and qs >= 1) else (4 if qs >= 1 else DEPTH)
                stv = (pvdA, pvdB, {}, nkt, qs, j, depth)
                for kt in range(nkt):
                    r = kt - 4 * qs
                    q0 = 128 * r if r > 0 else 0
                    sc = ps_sc.tile([128, 1024], f32, tag="sc")
                    # two heads as row tiles: (0,0) and (64,0)
                    for h, base in ((0, 0), (1, 64)):
                        nc.tensor.matmul(
                            sc[:, h * 512 + q0:(h + 1) * 512],
                            qkT[4][base:base + 64, kt * 128:(kt + 1) * 128],
                            qkT[j][base:base + 64,
                                   qs * SLAB + q0:(qs + 1) * SLAB],
                            start=True, stop=True,
                        )
                    p = pp.tile([128, 1024], bf16, tag="p")
                    if q0 == 0:
                        nc.scalar.activation(p[:], sc[:], AF.Exp, scale=float(SCALE))
                    else:
                        nc.scalar.activation(
                            p[:, q0:512], sc[:, q0:512], AF.Exp, scale=float(SCALE))
                        nc.scalar.activation(
                            p[:, 512 + q0:1024], sc[:, 512 + q0:1024],
                            AF.Exp, scale=float(SCALE))
                    if r >= 0:
                        # triangular mask on the diagonal 128x128 block
                        nc.vector.tensor_tensor(
                            p[:, q0:q0 + 128], p[:, q0:q0 + 128],
                            mask_sb[:], OP.mult)
                        nc.vector.tensor_tensor(
                            p[:, 512 + q0:512 + q0 + 128],
                            p[:, 512 + q0:512 + q0 + 128],
                            mask_sb[:], OP.mult)
                    stv[2][kt] = p
                    if kt >= depth:
                        emit_pv(stv, kt - depth)
                return stv

            def emit_B_norm(stv):
                pvdA, pvdB, probs, nkt, qs, j = stv[:6]
                qsl = slice(qs * SLAB, (qs + 1) * SLAB)
                with tc.high_priority():
                    rec = pn.tile([128, 512], f32, tag="rec")
                    nc.vector.reciprocal(rec[0:64, :], pvdA[64:128, :])
                    nc.vector.reciprocal(rec[64:128, :], pvdB[0:64, :])
                    nc.vector.tensor_tensor(
                        attnT[j][0:64, qsl], pvdA[0:64, :], rec[0:64, :],
                        OP.mult)
                    nc.vector.tensor_tensor(
                        attnT[j][64:128, qsl], pvdB[64:128, :],
                        rec[64:128, :], OP.mult)

            def emit_B_tail(stv):
                """Drain PVs + normalization; emitted after the next
                stream's head so its scores aren't stuck behind these in
                the PE queue."""
                for k0 in range(max(0, stv[3] - stv[6]), stv[3]):
                    emit_pv(stv, k0)
                emit_B_norm(stv)

            # ================= Phase C: out projection (one q-row) ======
            def emit_C_i(qs, ii):
                for i in (4 * qs + ii,):
                    last = (i == DT - 1)
                    ot = pc.tile([128, 2048], bf16, tag="ot")
                    for ns in range(4):
                        po = ps512.tile([128, 512], f32, tag="w")
                        for j in range(4):
                            nc.tensor.matmul(
                                po[:],
                                attnT[j][:, i * 128:(i + 1) * 128],
                                wo_sb[:, j * 2048 + ns * 512:j * 2048 + (ns + 1) * 512],
                                start=(j == 0), stop=(j == 3),
                            )
                        nc.vector.tensor_copy(ot[:, ns * 512:(ns + 1) * 512], po[:])
                        if last:
                            # final row: per-chunk DMAs on the idle HWDGE
                            # path shorten the post-PE drain tail
                            nc.sync.dma_start(
                                outp[i * 128:(i + 1) * 128,
                                     ns * 512:(ns + 1) * 512],
                                ot[:, ns * 512:(ns + 1) * 512])
                    if not last:
                        nc.gpsimd.dma_start(outp[i * 128:(i + 1) * 128, :], ot[:])

            # Interleaved waves: A(w+1) e-tiles woven between B(w) j-tiles
            # so PE priority alternates; C(w-1) rows backfill wave w.
            # B streams are software-pipelined across j: the next stream's
            # head (scores/exp) is emitted before the previous stream's
            # tail (drain PV + norm).
            EORD = (0, 4, 5, 1, 2, 3)
            for e in EORD:
                emit_A_e(0, e)
            pend = None  # pending B-stream tail
            for w in range(4):
                nxt = w + 1 if w < 3 else None
                def A(e):
                    if nxt is not None:
                        emit_A_e(nxt, e)
                prev = w - 1
                C = (lambda ii: emit_C_i(prev, ii)) if prev >= 0 else (lambda ii: None)

                def B(j):
                    nonlocal pend
                    stv = emit_B_head(w, j)
                    if pend is not None:
                        emit_B_tail(pend)
                    pend = stv
                B(0)
                A(0)
                B(1)
                A(4)
                C(0)
                B(2)
                A(5)
                A(1)
                C(1)
                B(3)
                A(2)
                A(3)
                C(2)
                C(3)
            # Epilogue: drain the last stream's PVs, then cover its norm
            # chain (4 serial DVE ops) with C(3) row-12's j=0..2 matmuls
            # before emitting the j=3 matmuls that depend on attnT[3].
            for k0 in range(max(0, pend[3] - pend[6]), pend[3]):
                emit_pv(pend, k0)
            i0 = 12
            # ns0/ns1 psum from ps512 (free immediately), ns2/ns3 from a
            # ps_sc buffer (free once the last exp has read its scores)
            ppo0 = ps512.tile([128, 512], f32, tag="w", name="ppo0")
            ppo1 = ps512.tile([128, 512], f32, tag="w", name="ppo1")
            ppo23 = ps_sc.tile([128, 1024], f32, tag="sc", name="ppo23")
            ppo = [ppo0[:], ppo1[:], ppo23[:, 0:512], ppo23[:, 512:1024]]
            for ns in range(4):
                po = ppo[ns]
                for j in range(3):
                    nc.tensor.matmul(
                        po,
                        attnT[j][:, i0 * 128:(i0 + 1) * 128],
                        wo_sb[:, j * 2048 + ns * 512:j * 2048 + (ns + 1) * 512],
                        start=(j == 0), stop=False,
                    )
            emit_B_norm(pend)
            ot0 = pc.tile([128, 2048], bf16, tag="ot")
            for ns in range(4):
                po = ppo[ns]
                nc.tensor.matmul(
                    po,
                    attnT[3][:, i0 * 128:(i0 + 1) * 128],
                    wo_sb[:, 3 * 2048 + ns * 512:3 * 2048 + (ns + 1) * 512],
                    start=False, stop=True,
                )
                nc.vector.tensor_copy(ot0[:, ns * 512:(ns + 1) * 512], po)
            nc.gpsimd.dma_start(outp[i0 * 128:(i0 + 1) * 128, :], ot0[:])
            for ii in range(1, 4):
                emit_C_i(3, ii)

    nc.finalize()
    return nc


def _host_inputs(x, sin, cos, w_qkv, w_out):
    bf = ml_dtypes.bfloat16
    sinT_np = np.concatenate([sin.T, sin.T], axis=0).astype(bf)  # [128, T]
    cosT_np = np.concatenate([cos.T, cos.T], axis=0).astype(bf)

    perm_np = np.zeros((128, 128), np.float32)
    for blk in range(2):
        for p in range(64):
            k = blk * 64 + ((p + 32) % 64)
            perm_np[k, blk * 64 + p] = -1.0 if p < 32 else 1.0
    perm_np = perm_np.astype(bf)
    id128_np = np.eye(128, dtype=np.float32).astype(bf)

    # [128, 128] lower-tri (inclusive) mask: key partition p valid for
    # local q col c when c >= p.
    cix = np.arange(128)[None, :]
    pix = np.arange(128)[:, None]
    mask_np = (cix >= pix).astype(np.float32).astype(bf)

    in_maps = []
    for c in range(NCORES):
        b, g = divmod(c, 4)
        cols = []
        for j in range(4):
            h1, h2 = 8 * g + j, 8 * g + 4 + j
            cols.append(w_qkv[:, 64 * h1:64 * h1 + 64])
            cols.append(w_qkv[:, 64 * h2:64 * h2 + 64])
        wq_np = np.concatenate(cols, axis=1).astype(bf)         # [D, 512]
        # e-major: [128, e=4, d=16, 128]
        wq_np = (wq_np.reshape(16, 128, 4, 128)
                 .transpose(1, 2, 0, 3).reshape(128, 16 * 512))
        wk_np = w_qkv[:, D + 128 * g: D + 128 * g + 128].astype(bf)  # k heads 2g,2g+1
        wk_np = wk_np.reshape(16, 128, 128).transpose(1, 0, 2).reshape(128, 16 * 128)
        wv_np = w_qkv[:, D + KVD + 128 * g: D + KVD + 128 * g + 128].astype(bf)
        wv_np = wv_np.reshape(16, 128, 128).transpose(1, 0, 2).reshape(128, 16 * 128)
        rows = []
        for j in range(4):
            h1, h2 = 8 * g + j, 8 * g + 4 + j
            rows.append(w_out[64 * h1:64 * h1 + 64, :])
            rows.append(w_out[64 * h2:64 * h2 + 64, :])
        wo_np = np.concatenate(rows, axis=0).astype(bf)         # [512, D]
        # [4, 128, 2048] -> [128, 4*2048]
        wo_np = wo_np.reshape(4, 128, D).transpose(1, 0, 2).reshape(128, 4 * D)
        in_maps.append({
            "xb": x[b].astype(bf),
            "wk": wk_np,
            "wq": wq_np,
            "wv": wv_np,
            "wo": wo_np,
            "sinT": sinT_np,
            "cosT": cosT_np,
            "perm": perm_np,
            "ident128": id128_np,
            "masks": mask_np,
        })
    return in_maps


def kernel(x, sin, cos, w_qkv, w_out, _trace=False):
    from concourse.bass_utils import run_bass_kernel_spmd

    if "nc" not in _CACHE:
        _CACHE["nc"] = _build()
    nc = _CACHE["nc"]

    in_maps = _host_inputs(
        np.asarray(x), np.asarray(sin), np.asarray(cos),
        np.asarray(w_qkv), np.asarray(w_out))
    res = run_bass_kernel_spmd(
        nc, in_maps, core_ids=list(range(NCORES)), trace=_trace)
    out = np.zeros((B, T, D), np.float32)
    for c in range(NCORES):
        b = c // 4
        out[b] += res.results[c]["outp"].astype(np.float32)
    if _trace:
        kernel.last_result = res
    return out

